# revision 21
# baseline (speedup 1.0000x reference)
"""Trainium2 Bass kernel for nn_DeletionLayer: out = where(mask, x @ W, x).

x: [200000, 1024] f32, deletion_weight: [1024, 1024] f32, mask: [200000] bool.

Sharding: data-parallel over the node axis across 8 NeuronCores. Each core
gets a uniform 25088-row (196 x 128) shard; core 7's shard overlaps core 6's
by 704 rows (identical rows recomputed, dropped at gather) so every core runs
the same program with full 128-row tiles only.

Fast path (used when deletion_weight is a constant matrix c*ones(dim,dim),
which is how the module initializes it): x @ W is rank-1, xw[i, :] ==
c * rowsum(x[i, :]), a constant per row. The kernel then never runs a matmul:

  per 512-row DMA group (4 x 128-row tiles, bf16):
    - ScalarE activation(Copy, scale=c, accum_out) computes s = c*rowsum per
      128-row tile while streaming x to a scratch buffer.
    - DVE multiplies s by the row mask (sm = s*m, 0 for unmasked rows).
    - DVE tensor_scalar computes out = x*(1-m) + sm per tile (per-partition
      scalars), i.e. where(mask, c*rowsum, x).
    - 1 MB DMAs in and out keep HBM at line rate; traffic is 2 bytes/elem
      each way (vs 10 for the f32+lhsT matmul path) so the kernel is
      DMA-bound at ~100 MB/core.

Fallback (general W): the original bf16 matmul kernel (16 matmuls per
128-row tile accumulating x @ W into PSUM, DVE copy_predicated select).
"""

from contextlib import ExitStack

import numpy as np

N_FULL = 200000
DIM = 1024
P = 128
KCH = DIM // P  # 8 contraction chunks (matmul path)
NCH = DIM // 512  # 2 PSUM-bank halves (matmul path)
R = 25088  # rows per core (196 full tiles)
T = R // P  # 196
N_CORES = 8
U = 14  # tiles per loop-body unroll (matmul path)
B = 7  # 128-row tiles per DMA group (fast path)
G = T // B  # 28 groups
ACT_TILES = 4  # tiles per group whose row-sum runs on ScalarE (rest on DVE)


def _ensure_profile_hook():
    """Make `antenv.axon_hooks` importable so that a BASS_TRACE=1 run of
    run_bass_kernel_spmd under axon can register NTFF profiling instead of
    crashing on the missing module. No-op when the module already exists."""
    import sys
    import types

    try:
        import antenv

        if hasattr(antenv, "axon_hooks") or "antenv.axon_hooks" in sys.modules:
            return
        mod = types.ModuleType("antenv.axon_hooks")
        _state = {"hook": None}
        mod.set_axon_ntff_profile_hook = lambda h: _state.__setitem__("hook", h)
        mod.get_axon_ntff_profile_hook = lambda: _state["hook"]
        sys.modules["antenv.axon_hooks"] = mod
        antenv.axon_hooks = mod
        try:
            from trn_agent_boot.trn_boot import _ntff_profile_via_ctypes

            hook = _ntff_profile_via_ctypes("/opt/axon/libaxon_pjrt.so")
            mod.set_axon_ntff_profile_hook(hook)
        except Exception:
            pass
    except Exception:
        pass


def _build_nc_fast():
    import concourse.bass as bass
    import concourse.tile as tile
    from concourse import bacc, mybir

    nc = bacc.Bacc("TRN2", target_bir_lowering=False, debug=False)

    x_dram = nc.dram_tensor("x", [R, DIM], mybir.dt.bfloat16, kind="ExternalInput")
    cm_dram = nc.dram_tensor("cmask", [P, T], mybir.dt.float32, kind="ExternalInput")
    iv_dram = nc.dram_tensor("invm", [P, T], mybir.dt.float32, kind="ExternalInput")
    o_dram = nc.dram_tensor("out", [R, DIM], mybir.dt.bfloat16, kind="ExternalOutput")

    with tile.TileContext(nc) as tc:
        with ExitStack() as ctx:
            wpool = ctx.enter_context(tc.tile_pool(name="w", bufs=1))
            xpool = ctx.enter_context(tc.tile_pool(name="x", bufs=6))
            opool = ctx.enter_context(tc.tile_pool(name="o", bufs=4))
            spool = ctx.enter_context(tc.tile_pool(name="s", bufs=4))

            # Tile (g, a) covers rows {g*B*P + p*B + a}: each partition owns B
            # consecutive DRAM rows per group, so every DMA moves contiguous
            # 14 KB runs per partition (7x bigger descriptors than the
            # row-block mapping). cmask[p, g*B+a] = c * mask of that row,
            # invm likewise 1 - mask.
            cmask = wpool.tile([P, T], mybir.dt.float32)
            nc.sync.dma_start(cmask[:], cm_dram[:])
            invm = wpool.tile([P, T], mybir.dt.float32)
            nc.sync.dma_start(invm[:], iv_dram[:])
            scratch = wpool.tile([P, B, DIM], mybir.dt.bfloat16)

            def emit_group(g):
                x_t = xpool.tile([P, B, DIM], mybir.dt.bfloat16, tag="x")
                nc.sync.dma_start(
                    x_t[:],
                    x_dram[bass.ts(g, B * P), :].rearrange("(p a) d -> p a d", p=P),
                )

                # sm[p] = mask * c * rowsum(x) for tile g*B+a: the scale
                # c*mask folds the select's masked branch into the reduction.
                # One [P,1] tile per a so each merge fires as soon as its own
                # reduce lands (no barrier on the whole group's reduces).
                sms = [
                    spool.tile([P, 1], mybir.dt.float32, tag=f"sm{a}", name=f"sm{a}")
                    for a in range(B)
                ]
                for a in range(ACT_TILES):
                    nc.scalar.activation(
                        scratch[:, a, :],
                        x_t[:, a, :],
                        mybir.ActivationFunctionType.Copy,
                        bias=0.0,
                        scale=cmask[:, bass.ds(g * B + a, 1)],
                        accum_out=sms[a][:],
                    )
                for a in range(ACT_TILES, B):
                    nc.vector.tensor_scalar(
                        scratch[:, a, :],
                        x_t[:, a, :],
                        cmask[:, bass.ds(g * B + a, 1)],
                        0.0,
                        mybir.AluOpType.mult,
                        mybir.AluOpType.add,
                        accum_out=sms[a][:],
                    )

                o_t = opool.tile([P, B, DIM], mybir.dt.bfloat16, tag="o")
                for a in range(B):
                    nc.vector.tensor_scalar(
                        o_t[:, a, :],
                        x_t[:, a, :],
                        invm[:, bass.ds(g * B + a, 1)],
                        sms[a][:],
                        mybir.AluOpType.mult,
                        mybir.AluOpType.add,
                    )

                # SWDGE (gpsimd) ring for the store: HWDGE DMAs are FIFO per
                # issuing engine, so keeping loads on the sync ring and stores
                # on the gpsimd ring lets them overlap.
                nc.gpsimd.dma_start(
                    o_dram[bass.ts(g, B * P), :].rearrange("(p a) d -> p a d", p=P),
                    o_t[:],
                )

            for g in range(G):
                emit_group(g)

    nc.compile()
    return nc


def _build_nc_matmul():
    import concourse.bass as bass
    import concourse.tile as tile
    from concourse import bacc, mybir

    n_loop = T // U
    nc = bacc.Bacc("TRN2", target_bir_lowering=False, debug=False)

    x_dram = nc.dram_tensor("x", [R, DIM], mybir.dt.float32, kind="ExternalInput")
    xt_dram = nc.dram_tensor(
        "xt", [T * DIM, P], mybir.dt.bfloat16, kind="ExternalInput"
    )
    w_dram = nc.dram_tensor("w", [DIM, DIM], mybir.dt.bfloat16, kind="ExternalInput")
    m_dram = nc.dram_tensor("mask", [P, T], mybir.dt.uint8, kind="ExternalInput")
    o_dram = nc.dram_tensor("out", [R, DIM], mybir.dt.float32, kind="ExternalOutput")

    with tile.TileContext(nc) as tc:
        with ExitStack() as ctx:
            wpool = ctx.enter_context(tc.tile_pool(name="w", bufs=1))
            xpool = ctx.enter_context(tc.tile_pool(name="x", bufs=3))
            xtpool = ctx.enter_context(tc.tile_pool(name="xt", bufs=3))
            pso_pool = ctx.enter_context(
                tc.tile_pool(name="psO", bufs=3, space="PSUM")
            )

            w_sb = wpool.tile([P, KCH, DIM], mybir.dt.bfloat16)
            nc.sync.dma_start(w_sb[:], w_dram.ap().rearrange("(c p) d -> p c d", p=P))
            m_all = wpool.tile([P, T], mybir.dt.uint8)
            nc.sync.dma_start(m_all[:], m_dram[:])

            def emit_tile(t):
                x_t = xpool.tile([P, DIM], mybir.dt.float32, tag="x")
                nc.sync.dma_start(x_t[:], x_dram[bass.ts(t, P), :])

                xT = xtpool.tile([P, KCH, P], mybir.dt.bfloat16, tag="xT")
                nc.sync.dma_start(
                    xT[:],
                    xt_dram[bass.ts(t, DIM), :].rearrange("(c i) j -> i c j", i=P),
                )

                psO = pso_pool.tile([P, DIM], mybir.dt.float32, tag="psO")
                for n in range(NCH):
                    for k in range(KCH):
                        nc.tensor.matmul(
                            psO[:, n * 512 : (n + 1) * 512],
                            xT[:, k, :],
                            w_sb[:, k, n * 512 : (n + 1) * 512],
                            start=(k == 0),
                            stop=(k == KCH - 1),
                        )

                nc.vector.copy_predicated(
                    x_t[:],
                    m_all[:, bass.ds(t, 1)].broadcast_to([P, DIM]),
                    psO[:],
                )
                nc.sync.dma_start(o_dram[bass.ts(t, P), :], x_t[:])

            with tc.For_i(0, n_loop, 1) as i:
                for j in range(U):
                    emit_tile(i * U + j)

    nc.compile()
    return nc


def _shard_starts(n):
    return [c * R for c in range(N_CORES - 1)] + [n - R]


def _core_map_fast(xs, ms, c):
    import ml_dtypes

    # column g*B+a of cmask/invm = mask of rows {g*B*P + p*B + a} (see
    # _build_nc_fast's tile mapping)
    m = ms.reshape(G, P, B).transpose(1, 0, 2).reshape(P, T).astype(np.float32)
    return {
        "x": xs.astype(ml_dtypes.bfloat16),
        "cmask": np.ascontiguousarray(np.float32(c) * m),
        "invm": np.ascontiguousarray(1.0 - m),
    }


def _core_map_matmul(xs, ms, w_in):
    import ml_dtypes

    # xt[t, c, i, j] = x[t*128 + j, c*128 + i] — lhsT blocks, bf16
    xt = (
        np.ascontiguousarray(xs.reshape(T, P, KCH, P).transpose(0, 2, 3, 1))
        .astype(ml_dtypes.bfloat16)
        .reshape(T * DIM, P)
    )
    return {
        "x": np.ascontiguousarray(xs),
        "xt": xt,
        "w": w_in,
        "mask": np.ascontiguousarray(ms.astype(np.uint8).reshape(T, P).T),
    }


_cached = {}


def _get_nc_fast(c):
    if "fast" not in _cached:
        _cached["fast"] = _build_nc_fast()
    return _cached["fast"]


def _get_nc_matmul():
    if "matmul" not in _cached:
        _cached["matmul"] = _build_nc_matmul()
    return _cached["matmul"]


def _prep(x, deletion_weight, mask):
    """Build (nc, in_maps) for the SPMD run. Fast path when W == c*ones."""
    import ml_dtypes

    x = np.asarray(x, dtype=np.float32)
    w = np.asarray(deletion_weight, dtype=np.float32)
    mask = np.asarray(mask)
    n = x.shape[0]
    assert n == N_FULL and x.shape[1] == DIM

    starts = _shard_starts(n)
    w_flat = w.ravel()
    if (w_flat == w_flat[0]).all():
        c = w_flat[0]
        nc = _get_nc_fast(c)
        in_maps = [
            _core_map_fast(x[r0 : r0 + R], mask[r0 : r0 + R], c) for r0 in starts
        ]
    else:
        nc = _get_nc_matmul()
        w_bf = w.astype(ml_dtypes.bfloat16)
        in_maps = [
            _core_map_matmul(x[r0 : r0 + R], mask[r0 : r0 + R], w_bf) for r0 in starts
        ]
    return nc, in_maps


def kernel(x, deletion_weight, mask):
    _ensure_profile_hook()
    from concourse import bass_utils

    n = np.asarray(x).shape[0]
    nc, in_maps = _prep(x, deletion_weight, mask)

    res = bass_utils.run_bass_kernel_spmd(nc, in_maps, core_ids=list(range(N_CORES)))

    out = np.empty((n, DIM), np.float32)
    starts = _shard_starts(n)
    for c in range(N_CORES - 1):
        out[starts[c] : starts[c] + R] = res.results[c]["out"]
    out[n - R :] = res.results[-1]["out"]
    return out


# revision 23
# speedup vs baseline: 1.0064x; 1.0064x over previous
"""Trainium2 Bass kernel for nn_DeletionLayer: out = where(mask, x @ W, x).

x: [200000, 1024] f32, deletion_weight: [1024, 1024] f32, mask: [200000] bool.

Sharding: data-parallel over the node axis across 8 NeuronCores. Each core
gets a uniform 25088-row (196 x 128) shard; core 7's shard overlaps core 6's
by 704 rows (identical rows recomputed, dropped at gather) so every core runs
the same program with full 128-row tiles only.

Fast path (used when deletion_weight is a constant matrix c*ones(dim,dim),
which is how the module initializes it): x @ W is rank-1, xw[i, :] ==
c * rowsum(x[i, :]), a constant per row. The kernel then never runs a matmul:

  per 512-row DMA group (4 x 128-row tiles, bf16):
    - ScalarE activation(Copy, scale=c, accum_out) computes s = c*rowsum per
      128-row tile while streaming x to a scratch buffer.
    - DVE multiplies s by the row mask (sm = s*m, 0 for unmasked rows).
    - DVE tensor_scalar computes out = x*(1-m) + sm per tile (per-partition
      scalars), i.e. where(mask, c*rowsum, x).
    - 1 MB DMAs in and out keep HBM at line rate; traffic is 2 bytes/elem
      each way (vs 10 for the f32+lhsT matmul path) so the kernel is
      DMA-bound at ~100 MB/core.

Fallback (general W): the original bf16 matmul kernel (16 matmuls per
128-row tile accumulating x @ W into PSUM, DVE copy_predicated select).
"""

from contextlib import ExitStack

import numpy as np

N_FULL = 200000
DIM = 1024
P = 128
KCH = DIM // P  # 8 contraction chunks (matmul path)
NCH = DIM // 512  # 2 PSUM-bank halves (matmul path)
R = 25088  # rows per core (196 full tiles)
T = R // P  # 196
N_CORES = 8
U = 14  # tiles per loop-body unroll (matmul path)
B = 7  # 128-row tiles per DMA group (fast path)
G = T // B  # 28 groups
ACT_TILES = 4  # tiles per group whose row-sum runs on ScalarE (rest on DVE)


def _ensure_profile_hook():
    """Make `antenv.axon_hooks` importable so that a BASS_TRACE=1 run of
    run_bass_kernel_spmd under axon can register NTFF profiling instead of
    crashing on the missing module. No-op when the module already exists."""
    import sys
    import types

    try:
        import antenv

        if hasattr(antenv, "axon_hooks") or "antenv.axon_hooks" in sys.modules:
            return
        mod = types.ModuleType("antenv.axon_hooks")
        _state = {"hook": None}
        mod.set_axon_ntff_profile_hook = lambda h: _state.__setitem__("hook", h)
        mod.get_axon_ntff_profile_hook = lambda: _state["hook"]
        sys.modules["antenv.axon_hooks"] = mod
        antenv.axon_hooks = mod
        try:
            from trn_agent_boot.trn_boot import _ntff_profile_via_ctypes

            hook = _ntff_profile_via_ctypes("/opt/axon/libaxon_pjrt.so")
            mod.set_axon_ntff_profile_hook(hook)
        except Exception:
            pass
    except Exception:
        pass


def _build_nc_fast():
    import concourse.bass as bass
    import concourse.tile as tile
    from concourse import bacc, mybir

    nc = bacc.Bacc("TRN2", target_bir_lowering=False, debug=False)

    x_dram = nc.dram_tensor("x", [R, DIM], mybir.dt.bfloat16, kind="ExternalInput")
    cm_dram = nc.dram_tensor("cmask", [P, T], mybir.dt.float32, kind="ExternalInput")
    iv_dram = nc.dram_tensor("invm", [P, T], mybir.dt.float32, kind="ExternalInput")
    o_dram = nc.dram_tensor("out", [R, DIM], mybir.dt.bfloat16, kind="ExternalOutput")

    with tile.TileContext(nc) as tc:
        with ExitStack() as ctx:
            wpool = ctx.enter_context(tc.tile_pool(name="w", bufs=1))
            xpool = ctx.enter_context(tc.tile_pool(name="x", bufs=6))
            opool = ctx.enter_context(tc.tile_pool(name="o", bufs=4))
            spool = ctx.enter_context(tc.tile_pool(name="s", bufs=4))

            # Tile (g, a) covers rows {g*B*P + p*B + a}: each partition owns B
            # consecutive DRAM rows per group, so every DMA moves contiguous
            # 14 KB runs per partition (7x bigger descriptors than the
            # row-block mapping). cmask[p, g*B+a] = c * mask of that row,
            # invm likewise 1 - mask.
            cmask = wpool.tile([P, T], mybir.dt.float32)
            nc.gpsimd.dma_start(cmask[:], cm_dram[:])
            invm = wpool.tile([P, T], mybir.dt.float32)
            nc.gpsimd.dma_start(invm[:], iv_dram[:])
            scratch = wpool.tile([P, B, DIM], mybir.dt.bfloat16)

            def emit_group(g):
                x_t = xpool.tile([P, B, DIM], mybir.dt.bfloat16, tag="x")
                nc.sync.dma_start(
                    x_t[:],
                    x_dram[bass.ts(g, B * P), :].rearrange("(p a) d -> p a d", p=P),
                )

                # sm[p] = mask * c * rowsum(x) for tile g*B+a: the scale
                # c*mask folds the select's masked branch into the reduction.
                # One [P,1] tile per a so each merge fires as soon as its own
                # reduce lands (no barrier on the whole group's reduces).
                sms = [
                    spool.tile([P, 1], mybir.dt.float32, tag=f"sm{a}", name=f"sm{a}")
                    for a in range(B)
                ]
                for a in range(ACT_TILES):
                    nc.scalar.activation(
                        scratch[:, a, :],
                        x_t[:, a, :],
                        mybir.ActivationFunctionType.Copy,
                        bias=0.0,
                        scale=cmask[:, bass.ds(g * B + a, 1)],
                        accum_out=sms[a][:],
                    )
                for a in range(ACT_TILES, B):
                    nc.vector.tensor_scalar(
                        scratch[:, a, :],
                        x_t[:, a, :],
                        cmask[:, bass.ds(g * B + a, 1)],
                        0.0,
                        mybir.AluOpType.mult,
                        mybir.AluOpType.add,
                        accum_out=sms[a][:],
                    )

                # Merges split DVE/GpSimd so the max engine load stays well
                # under the DMA pace even when engine clocks throttle.
                o_t = opool.tile([P, B, DIM], mybir.dt.bfloat16, tag="o")
                for a in range(B):
                    eng = nc.vector if a < ACT_TILES else nc.gpsimd
                    eng.tensor_scalar(
                        o_t[:, a, :],
                        x_t[:, a, :],
                        invm[:, bass.ds(g * B + a, 1)],
                        sms[a][:],
                        mybir.AluOpType.mult,
                        mybir.AluOpType.add,
                    )

                # SWDGE (gpsimd) ring for the store: HWDGE DMAs are FIFO per
                # issuing engine, so keeping loads on the sync ring and stores
                # on the gpsimd ring lets them overlap.
                nc.gpsimd.dma_start(
                    o_dram[bass.ts(g, B * P), :].rearrange("(p a) d -> p a d", p=P),
                    o_t[:],
                )

            for g in range(G):
                emit_group(g)

    nc.compile()
    return nc


def _build_nc_matmul():
    import concourse.bass as bass
    import concourse.tile as tile
    from concourse import bacc, mybir

    n_loop = T // U
    nc = bacc.Bacc("TRN2", target_bir_lowering=False, debug=False)

    x_dram = nc.dram_tensor("x", [R, DIM], mybir.dt.float32, kind="ExternalInput")
    xt_dram = nc.dram_tensor(
        "xt", [T * DIM, P], mybir.dt.bfloat16, kind="ExternalInput"
    )
    w_dram = nc.dram_tensor("w", [DIM, DIM], mybir.dt.bfloat16, kind="ExternalInput")
    m_dram = nc.dram_tensor("mask", [P, T], mybir.dt.uint8, kind="ExternalInput")
    o_dram = nc.dram_tensor("out", [R, DIM], mybir.dt.float32, kind="ExternalOutput")

    with tile.TileContext(nc) as tc:
        with ExitStack() as ctx:
            wpool = ctx.enter_context(tc.tile_pool(name="w", bufs=1))
            xpool = ctx.enter_context(tc.tile_pool(name="x", bufs=3))
            xtpool = ctx.enter_context(tc.tile_pool(name="xt", bufs=3))
            pso_pool = ctx.enter_context(
                tc.tile_pool(name="psO", bufs=3, space="PSUM")
            )

            w_sb = wpool.tile([P, KCH, DIM], mybir.dt.bfloat16)
            nc.sync.dma_start(w_sb[:], w_dram.ap().rearrange("(c p) d -> p c d", p=P))
            m_all = wpool.tile([P, T], mybir.dt.uint8)
            nc.sync.dma_start(m_all[:], m_dram[:])

            def emit_tile(t):
                x_t = xpool.tile([P, DIM], mybir.dt.float32, tag="x")
                nc.sync.dma_start(x_t[:], x_dram[bass.ts(t, P), :])

                xT = xtpool.tile([P, KCH, P], mybir.dt.bfloat16, tag="xT")
                nc.sync.dma_start(
                    xT[:],
                    xt_dram[bass.ts(t, DIM), :].rearrange("(c i) j -> i c j", i=P),
                )

                psO = pso_pool.tile([P, DIM], mybir.dt.float32, tag="psO")
                for n in range(NCH):
                    for k in range(KCH):
                        nc.tensor.matmul(
                            psO[:, n * 512 : (n + 1) * 512],
                            xT[:, k, :],
                            w_sb[:, k, n * 512 : (n + 1) * 512],
                            start=(k == 0),
                            stop=(k == KCH - 1),
                        )

                nc.vector.copy_predicated(
                    x_t[:],
                    m_all[:, bass.ds(t, 1)].broadcast_to([P, DIM]),
                    psO[:],
                )
                nc.sync.dma_start(o_dram[bass.ts(t, P), :], x_t[:])

            with tc.For_i(0, n_loop, 1) as i:
                for j in range(U):
                    emit_tile(i * U + j)

    nc.compile()
    return nc


def _shard_starts(n):
    return [c * R for c in range(N_CORES - 1)] + [n - R]


def _core_map_fast(xs, ms, c):
    import ml_dtypes

    # column g*B+a of cmask/invm = mask of rows {g*B*P + p*B + a} (see
    # _build_nc_fast's tile mapping)
    m = ms.reshape(G, P, B).transpose(1, 0, 2).reshape(P, T).astype(np.float32)
    return {
        "x": xs.astype(ml_dtypes.bfloat16),
        "cmask": np.ascontiguousarray(np.float32(c) * m),
        "invm": np.ascontiguousarray(1.0 - m),
    }


def _core_map_matmul(xs, ms, w_in):
    import ml_dtypes

    # xt[t, c, i, j] = x[t*128 + j, c*128 + i] — lhsT blocks, bf16
    xt = (
        np.ascontiguousarray(xs.reshape(T, P, KCH, P).transpose(0, 2, 3, 1))
        .astype(ml_dtypes.bfloat16)
        .reshape(T * DIM, P)
    )
    return {
        "x": np.ascontiguousarray(xs),
        "xt": xt,
        "w": w_in,
        "mask": np.ascontiguousarray(ms.astype(np.uint8).reshape(T, P).T),
    }


_cached = {}


def _get_nc_fast(c):
    if "fast" not in _cached:
        _cached["fast"] = _build_nc_fast()
    return _cached["fast"]


def _get_nc_matmul():
    if "matmul" not in _cached:
        _cached["matmul"] = _build_nc_matmul()
    return _cached["matmul"]


def _prep(x, deletion_weight, mask):
    """Build (nc, in_maps) for the SPMD run. Fast path when W == c*ones."""
    import ml_dtypes

    x = np.asarray(x, dtype=np.float32)
    w = np.asarray(deletion_weight, dtype=np.float32)
    mask = np.asarray(mask)
    n = x.shape[0]
    assert n == N_FULL and x.shape[1] == DIM

    starts = _shard_starts(n)
    w_flat = w.ravel()
    if (w_flat == w_flat[0]).all():
        c = w_flat[0]
        nc = _get_nc_fast(c)
        in_maps = [
            _core_map_fast(x[r0 : r0 + R], mask[r0 : r0 + R], c) for r0 in starts
        ]
    else:
        nc = _get_nc_matmul()
        w_bf = w.astype(ml_dtypes.bfloat16)
        in_maps = [
            _core_map_matmul(x[r0 : r0 + R], mask[r0 : r0 + R], w_bf) for r0 in starts
        ]
    return nc, in_maps


def kernel(x, deletion_weight, mask):
    _ensure_profile_hook()
    from concourse import bass_utils

    n = np.asarray(x).shape[0]
    nc, in_maps = _prep(x, deletion_weight, mask)

    res = bass_utils.run_bass_kernel_spmd(nc, in_maps, core_ids=list(range(N_CORES)))

    out = np.empty((n, DIM), np.float32)
    starts = _shard_starts(n)
    for c in range(N_CORES - 1):
        out[starts[c] : starts[c] + R] = res.results[c]["out"]
    out[n - R :] = res.results[-1]["out"]
    return out
